# revision 1
# baseline (speedup 1.0000x reference)
"""GCGRU cell (order-2 graph diffusion GRU) Trainium2 Bass kernel.

Strategy: data-parallel over batch (B=16 -> 2 batches per core x 8 cores).
Per core, activations are kept node-major ([node-chunk partitions x (b,c)
columns], fp16) so the graph-diffusion matmuls (contract over the node dim)
run with adj^T tiles as the PE-stationary operand, streamed from HBM and
accumulated over n-chunks in PSUM. The node dim is zero-padded to 4096 so
every tile is a full 128 partitions / 128 columns (fast weight load). adj^T
is host-retiled partition-major so each slab DMA is one dense transfer with
multi-KB contiguous runs per partition.

The gates share one diffusion of z=[x;h]; since z1=A z already contains A x,
the candidate path only diffuses r*h (128 batch-channel columns), using r*h as
the PE-stationary operand and adj as the 512-wide moving operand, producing
batch-major outputs that feed the candidate conv directly. The final candidate
conv + tanh + u*h+(1-u)*c combine is fused into the last diffusion's PSUM
group loop so the kernel tail is one group deep. sigmoid/tanh on ScalarE.
All input casts/layout transforms are done on host in kernel().
"""

import numpy as np

import concourse.bass as bass
from concourse import bacc
import concourse.mybir as mybir
import concourse.tile as tile
from concourse.bass_utils import run_bass_kernel_spmd

# problem constants
B, D_IN, D_H, NN = 16, 32, 64, 4000
NCORES = 8
B_LOC = B // NCORES          # batches per core
C = D_IN + D_H               # 96 channels into each gate conv
BC = B_LOC * C               # node-major column count (b-major: [b0 c96 | b1 c96])
BH = B_LOC * D_H             # stacked batch-hidden rows (128)
NP = 4096                    # node dim padded to a multiple of 128

F16 = mybir.dt.float16
F32 = mybir.dt.float32
CHUNK = 128


def build_program(npad=NP, nn=NN, mg=4, jb=8, nsl=512):
    """Build the single-core Bass program (same program runs SPMD on 8 cores).

    npad: padded node count; mg: m-chunks per PSUM group; jb: n-chunk blocks
    merged per slab DMA; nsl: node slice width for conv/elementwise loops.
    """
    chunk = CHUNK
    nch = npad // chunk          # node chunks
    ngrp = nch // mg             # psum groups per diffusion stage
    nsli = npad // nsl           # conv node slices
    assert nch % mg == 0 and npad % nsl == 0 and nch % jb == 0
    assert nsl == mg * chunk     # fused consumer: conv slice == psum group band
    assert BH == chunk

    nc = bacc.Bacc("TRN2", target_bir_lowering=False, debug=False)

    # ---- DRAM I/O (all host-prepped layouts) ----
    # at_t[g, p, j, :] = adjT[j*128+p, g*mg*128:(g+1)*mg*128]  (partition-major:
    # per partition, all n-chunk blocks of a group band are contiguous)
    at_d = nc.dram_tensor("at", [ngrp, chunk, nch, mg * chunk], F16,
                          kind="ExternalInput").ap()
    zt_d = nc.dram_tensor("zt", [npad, BC], F16, kind="ExternalInput").ap()
    xh_d = nc.dram_tensor("xh", [B_LOC, C, npad], F16, kind="ExternalInput").ap()
    h_d = nc.dram_tensor("h", [B_LOC, D_H, npad], F16, kind="ExternalInput").ap()
    wf_d = nc.dram_tensor("wf", [3, C, D_H], F16, kind="ExternalInput").ap()
    wu_d = nc.dram_tensor("wu", [3, C, D_H], F16, kind="ExternalInput").ap()
    # candidate weights: x rows per diffusion order, and batch-duplicated rh rows
    wcx_d = nc.dram_tensor("wcx", [3, D_IN, D_H], F16, kind="ExternalInput").ap()
    wcrh_d = nc.dram_tensor("wcrh", [3, BH, D_H], F16, kind="ExternalInput").ap()
    bf_d = nc.dram_tensor("bf", [BH, 1], F32, kind="ExternalInput").ap()
    bu_d = nc.dram_tensor("bu", [BH, 1], F32, kind="ExternalInput").ap()
    bc_d = nc.dram_tensor("bcb", [BH, 1], F32, kind="ExternalInput").ap()
    id_d = nc.dram_tensor("idm", [chunk, chunk], F16, kind="ExternalInput").ap()
    out_d = nc.dram_tensor("out", [B_LOC, D_H, nn], F32, kind="ExternalOutput").ap()

    with tile.TileContext(nc) as tc:
        _body(tc, locals())
    nc.compile()
    return nc


def _body(tc, aps):
    nc = tc.nc
    npad, nn, chunk, mg, jb, nsl = (aps[k] for k in
                                    ("npad", "nn", "chunk", "mg", "jb", "nsl"))
    nch, ngrp, nsli = aps["nch"], aps["ngrp"], aps["nsli"]
    at_d, zt_d, xh_d, h_d = aps["at_d"], aps["zt_d"], aps["xh_d"], aps["h_d"]
    wf_d, wu_d, wcx_d, wcrh_d = (
        aps["wf_d"], aps["wu_d"], aps["wcx_d"], aps["wcrh_d"])
    bf_d, bu_d, bc_d, id_d, out_d = (
        aps["bf_d"], aps["bu_d"], aps["bc_d"], aps["id_d"], aps["out_d"])

    SIG = mybir.ActivationFunctionType.Sigmoid
    TANH = mybir.ActivationFunctionType.Tanh

    with (
        tc.tile_pool(name="const", bufs=1) as cpool,       # persistent small tiles
        tc.tile_pool(name="perst", bufs=1) as ppool,       # persistent activations
        tc.tile_pool(name="nmrot", bufs=2) as nmpool,      # rotating node-major tensors
        tc.tile_pool(name="cmrot", bufs=4) as cmpool,      # rotating channel-major tensors
        tc.tile_pool(name="slab", bufs=5) as slpool,       # adj slabs
        tc.tile_pool(name="psum", bufs=8, space="PSUM") as pspool,
        tc.tile_pool(name="stage", bufs=2) as stpool,      # small staging tiles
    ):
        # ---- persistent loads ----
        idm = cpool.tile([chunk, chunk], F16, tag="idm")
        nc.sync.dma_start(out=idm[:], in_=id_d[:])
        wf_sb = [cpool.tile([C, D_H], F16, tag=f"wf{k}", name=f"wf{k}")
                 for k in range(3)]
        wu_sb = [cpool.tile([C, D_H], F16, tag=f"wu{k}", name=f"wu{k}")
                 for k in range(3)]
        wcx_sb = [cpool.tile([D_IN, D_H], F16, tag=f"wcx{k}", name=f"wcx{k}")
                  for k in range(3)]
        wcrh_sb = [cpool.tile([BH, D_H], F16, tag=f"wcrh{k}", name=f"wcrh{k}")
                   for k in range(3)]
        for k in range(3):
            nc.scalar.dma_start(out=wf_sb[k][:], in_=wf_d[k])
            nc.scalar.dma_start(out=wu_sb[k][:], in_=wu_d[k])
            nc.scalar.dma_start(out=wcx_sb[k][:], in_=wcx_d[k])
            nc.scalar.dma_start(out=wcrh_sb[k][:], in_=wcrh_d[k])
        bf_sb = cpool.tile([BH, 1], F32, tag="bf")
        nc.sync.dma_start(out=bf_sb[:], in_=bf_d[:])
        bu_sb = cpool.tile([BH, 1], F32, tag="bu")
        nc.sync.dma_start(out=bu_sb[:], in_=bu_d[:])
        bc_sb = cpool.tile([BH, 1], F32, tag="bc")
        nc.sync.dma_start(out=bc_sb[:], in_=bc_d[:])

        # node-major [x;h]: one tile, chunk j occupies cols [j*BC, (j+1)*BC)
        # (rotating pool: ztT is dead after the first diffusion, z2T reuses it)
        ztT = nmpool.tile([chunk, nch * BC], F16, tag="nm", name="ztT")
        nc.sync.dma_start(
            out=ztT[:, :].rearrange("p (j f) -> p j f", j=nch),
            in_=zt_d[:, :].rearrange("(j p) f -> p j f", p=chunk))

        xh_sb = [ppool.tile([C, npad], F16, tag=f"xh{b}", name=f"xh{b}")
                 for b in range(B_LOC)]
        for b in range(B_LOC):
            nc.scalar.dma_start(out=xh_sb[b][:], in_=xh_d[b])
        # batch-stacked [b0 rows 0:64 | b1 rows 64:128]
        h_st = ppool.tile([BH, npad], F16, tag="h_st")
        for b in range(B_LOC):
            nc.scalar.dma_start(out=h_st[b * D_H:(b + 1) * D_H, :], in_=h_d[b])
        u_st = ppool.tile([BH, npad], F16, tag="u_st")
        rh_st = ppool.tile([BH, npad], F16, tag="rh_st")

        # ---- helpers ----
        def load_slab(g, jB):
            # two triggers per slab, one per HWDGE ring (SP + ACT), so both
            # trigger queues and transfer paths run in parallel
            slab = slpool.tile([chunk, jb * mg * chunk], F16, tag="slab",
                               name="slab")
            h1 = jb // 2
            eng2 = nc.scalar
            nc.sync.dma_start(
                out=slab[:, 0:h1 * mg * chunk].rearrange(
                    "p (j m) -> p j m", j=h1),
                in_=at_d[g, :, jB * jb: jB * jb + h1, :])
            eng2.dma_start(
                out=slab[:, h1 * mg * chunk:].rearrange(
                    "p (j m) -> p j m", j=jb - h1),
                in_=at_d[g, :, jB * jb + h1:(jB + 1) * jb, :])
            return slab

        def diffusion_sa(src, dst):
            """dst = A @ src, node-major -> node-major (adj stationary)."""
            for g in range(ngrp):
                pss = [pspool.tile([chunk, BC], F32, tag="ps", name=f"psd{mi}")
                       for mi in range(mg)]
                for jB in range(nch // jb):
                    slab = load_slab(g, jB)
                    for jj in range(jb):
                        j = jB * jb + jj
                        for mi in range(mg):
                            nc.tensor.matmul(
                                pss[mi][:, :],
                                lhsT=slab[:, (jj * mg + mi) * chunk:
                                          (jj * mg + mi + 1) * chunk],
                                rhs=src[:, j * BC:(j + 1) * BC],
                                start=(j == 0), stop=(j == nch - 1))
                for mi in range(mg):
                    m = g * mg + mi
                    nc.vector.tensor_copy(
                        out=dst[:, m * BC:(m + 1) * BC], in_=pss[mi][:, :])

        def diffusion_sz(src_nm, dst_bm, consumer=None):
            """dst_bm[128 bc, m] = (A @ src)^T with src (node-major [n, 128bc])
            stationary and adj moving. Optionally calls consumer(g) after the
            group band [g*nsl, (g+1)*nsl) of dst_bm is available."""
            for g in range(ngrp):
                psc = pspool.tile([BH, mg * chunk], F32, tag="ps", name="psz")
                for jB in range(nch // jb):
                    slab = load_slab(g, jB)
                    for jj in range(jb):
                        j = jB * jb + jj
                        nc.tensor.matmul(
                            psc[:, :],
                            lhsT=src_nm[:, j * chunk:(j + 1) * chunk],
                            rhs=slab[:, jj * mg * chunk:(jj + 1) * mg * chunk],
                            start=(j == 0), stop=(j == nch - 1))
                nc.vector.tensor_copy(
                    out=dst_bm[:, g * mg * chunk:(g + 1) * mg * chunk],
                    in_=psc[:, :])
                if consumer is not None:
                    consumer(g)

        def to_channel_major(src_nm):
            """node-major [chunk, nch*BC] fp16 -> per-batch channel-major [C, npad]."""
            cms = [cmpool.tile([C, npad], F16, tag="cm", name=f"cm{b}")
                   for b in range(B_LOC)]
            for b in range(B_LOC):
                for j in range(nch):
                    pt = pspool.tile([C, chunk], F16, tag="ps")
                    nc.tensor.transpose(
                        pt[:, :],
                        src_nm[:, j * BC + b * C: j * BC + (b + 1) * C],
                        idm[:, :])
                    nc.vector.tensor_copy(
                        out=cms[b][:, j * chunk:(j + 1) * chunk], in_=pt[:, :])
            return cms

        # ---- gates path: z1 = A z, z2 = A z1 ----
        z1T = nmpool.tile([chunk, nch * BC], F16, tag="nm")
        diffusion_sa(ztT, z1T)
        z2T = nmpool.tile([chunk, nch * BC], F16, tag="nm")
        diffusion_sa(z1T, z2T)

        z1cm = to_channel_major(z1T)
        z2cm = to_channel_major(z2T)

        # gate convs: r and u, batch-stacked in PSUM partitions
        # (rhT: node-major r*h, filled per band inside the loop)
        rhT = ppool.tile([chunk, nch * BH], F16, tag="rhT")
        for s in range(nsli):
            sl = slice(s * nsl, (s + 1) * nsl)
            psf = pspool.tile([BH, nsl], F32, tag="ps", name="psf")
            psu = pspool.tile([BH, nsl], F32, tag="ps", name="psu")
            for b in range(B_LOC):
                rows = slice(b * D_H, (b + 1) * D_H)
                feats = (xh_sb[b][:, sl], z1cm[b][:, sl], z2cm[b][:, sl])
                for k in range(3):
                    nc.tensor.matmul(psf[rows, :], lhsT=wf_sb[k], rhs=feats[k],
                                     start=(k == 0), stop=(k == 2))
                for k in range(3):
                    nc.tensor.matmul(psu[rows, :], lhsT=wu_sb[k], rhs=feats[k],
                                     start=(k == 0), stop=(k == 2))
            rst = stpool.tile([BH, nsl], F16, tag="rst")
            nc.scalar.activation(rst[:, :], psf[:, :], SIG, bias=bf_sb[:, :])
            nc.vector.tensor_mul(out=rh_st[:, sl], in0=rst[:, :],
                                 in1=h_st[:, sl])
            nc.scalar.activation(u_st[:, sl], psu[:, :], SIG, bias=bu_sb[:, :])
            # rhT transposes for this node band, so the candidate diffusion
            # can start as soon as the band is ready
            for b in range(B_LOC):
                rows = slice(b * D_H, (b + 1) * D_H)
                for j in range(s * nsl // chunk, (s + 1) * nsl // chunk):
                    pt = pspool.tile([chunk, D_H], F16, tag="ps", name="ptr")
                    nc.tensor.transpose(
                        pt[:, :], rh_st[rows, j * chunk:(j + 1) * chunk],
                        idm[rows, rows])
                    nc.vector.tensor_copy(
                        out=rhT[:, j * BH + b * D_H: j * BH + (b + 1) * D_H],
                        in_=pt[:, :])

        zc1_bm = ppool.tile([BH, npad], F16, tag="zc1bm")
        diffusion_sz(rhT, zc1_bm)

        zc1T = ppool.tile([chunk, nch * BH], F16, tag="zc1T")
        for j in range(nch):
            pt = pspool.tile([chunk, chunk], F16, tag="ps")
            nc.tensor.transpose(pt[:, :],
                                zc1_bm[:, j * chunk:(j + 1) * chunk], idm[:, :])
            nc.vector.tensor_copy(
                out=zc1T[:, j * chunk:(j + 1) * chunk], in_=pt[:, :])

        zc2_bm = ppool.tile([BH, npad], F16, tag="zc2bm")

        def consumer(s):
            # candidate conv for node band s, then out = c + u*(h-c)
            sl = slice(s * nsl, (s + 1) * nsl)
            psc2 = pspool.tile([BH, nsl], F32, tag="ps", name="psc2")
            for b in range(B_LOC):
                rows = slice(b * D_H, (b + 1) * D_H)
                terms = ((wcx_sb[0], xh_sb[b][0:D_IN, sl]),
                         (wcx_sb[1], z1cm[b][0:D_IN, sl]),
                         (wcx_sb[2], z2cm[b][0:D_IN, sl]),
                         (wcrh_sb[0][rows, :], rh_st[rows, sl]),
                         (wcrh_sb[1][rows, :], zc1_bm[rows, sl]),
                         (wcrh_sb[2][rows, :], zc2_bm[rows, sl]))
                for k, (wt, rhs) in enumerate(terms):
                    nc.tensor.matmul(psc2[rows, :], lhsT=wt, rhs=rhs,
                                     start=(k == 0), stop=(k == len(terms) - 1))
            cst = stpool.tile([BH, nsl], F32, tag="cst")
            nc.scalar.activation(cst[:, :], psc2[:, :], TANH, bias=bc_sb[:, :])
            t1 = stpool.tile([BH, nsl], F32, tag="t1")
            nc.vector.tensor_sub(out=t1[:, :], in0=h_st[:, sl], in1=cst[:, :])
            nc.vector.tensor_mul(out=t1[:, :], in0=u_st[:, sl], in1=t1[:, :])
            ost = stpool.tile([BH, nsl], F32, tag="ost")
            nc.vector.tensor_add(out=ost[:, :], in0=cst[:, :], in1=t1[:, :])
            w = min(nsl, nn - s * nsl)
            if w > 0:
                for b in range(B_LOC):
                    nc.scalar.dma_start(
                        out=out_d[b][:, s * nsl: s * nsl + w],
                        in_=ost[b * D_H:(b + 1) * D_H, 0:w])

        diffusion_sz(zc1T, zc2_bm, consumer=consumer)


# ---- host-side driver ----
_CACHED_NC = None
TRACE = False           # set True (e.g. from test.py) to capture an NTFF profile
TRACE_DIR = None
LAST_RESULTS = None     # BassKernelResults of the most recent kernel() call


def _host_prep(x, h, adj, Wf, bf, Wu, bu, Wc, bc, npad=NP, nn=NN, mg=4):
    """Shard + cast + layout inputs for the 8 cores. Returns list of in_maps."""
    chunk = CHUNK
    nch = npad // chunk
    ngrp = nch // mg
    # adj^T zero-padded to [npad, npad], retiled partition-major per group band
    at = np.zeros((npad, npad), dtype=np.float16)
    at[:nn, :nn] = adj.T.astype(np.float16)
    at_t = np.ascontiguousarray(
        at.reshape(nch, chunk, ngrp, mg * chunk).transpose(2, 1, 0, 3))
    idm = np.eye(chunk, dtype=np.float16)

    def wsplit(W):
        WT = W.T.astype(np.float16)                            # [3C, D_H]
        return np.ascontiguousarray(WT.reshape(3, C, D_H))

    wf3, wu3, wc3 = wsplit(Wf), wsplit(Wu), wsplit(Wc)
    wcx3 = np.ascontiguousarray(wc3[:, :D_IN])                 # [3, D_IN, D_H]
    wcrh = wc3[:, D_IN:]                                       # [3, D_H, D_H]
    wcrh3 = np.ascontiguousarray(
        np.concatenate([wcrh] * B_LOC, axis=1))                # [3, BH, D_H]

    def bstack(v):
        return np.concatenate([v] * B_LOC).reshape(BH, 1).astype(np.float32)

    shared = {
        "wf": wf3, "wu": wu3, "wcx": wcx3, "wcrh": wcrh3,
        "bf": bstack(bf), "bu": bstack(bu), "bcb": bstack(bc),
        "idm": idm, "at": at_t,
    }
    xh = np.concatenate([x, h], axis=1).astype(np.float16)     # [B, C, nn]
    xh_p = np.zeros((B, C, npad), dtype=np.float16)
    xh_p[:, :, :nn] = xh
    h_p = np.zeros((B, D_H, npad), dtype=np.float16)
    h_p[:, :, :nn] = h.astype(np.float16)
    in_maps = []
    for core in range(NCORES):
        bs = slice(core * B_LOC, (core + 1) * B_LOC)
        xh_c = xh_p[bs]                                        # [B_LOC, C, npad]
        zt_c = np.ascontiguousarray(
            xh_c.transpose(2, 0, 1).reshape(npad, B_LOC * C))
        in_maps.append(dict(shared, zt=zt_c,
                            xh=np.ascontiguousarray(xh_c),
                            h=np.ascontiguousarray(h_p[bs])))
    return in_maps


def kernel(**inputs):
    global _CACHED_NC, LAST_RESULTS
    inputs = {k: np.asarray(v) for k, v in inputs.items()}
    if _CACHED_NC is None:
        _CACHED_NC = build_program()
    in_maps = _host_prep(**inputs)
    kw = {}
    if TRACE:
        kw = dict(trace=True, tmpdir=TRACE_DIR)
    res = run_bass_kernel_spmd(_CACHED_NC, in_maps,
                               core_ids=list(range(NCORES)), **kw)
    LAST_RESULTS = res
    outs = [res.results[i]["out"] for i in range(NCORES)]
    return np.concatenate(outs, axis=0).astype(np.float32)


if __name__ == "__main__":
    rng = np.random.default_rng(0)
    ins = {
        "x": rng.standard_normal((B, D_IN, NN), dtype=np.float32),
        "h": rng.standard_normal((B, D_H, NN), dtype=np.float32),
        "adj": rng.random((NN, NN), dtype=np.float32) / NN,
        "Wf": rng.standard_normal((D_H, 3 * C), dtype=np.float32) * 0.05,
        "Wu": rng.standard_normal((D_H, 3 * C), dtype=np.float32) * 0.05,
        "Wc": rng.standard_normal((D_H, 3 * C), dtype=np.float32) * 0.05,
        "bf": rng.standard_normal(D_H).astype(np.float32) * 0.05,
        "bu": rng.standard_normal(D_H).astype(np.float32) * 0.05,
        "bc": rng.standard_normal(D_H).astype(np.float32) * 0.05,
    }
    out = kernel(**ins)
    print(out.shape, out.dtype)



# revision 9
# speedup vs baseline: 1.4502x; 1.4502x over previous
"""GCGRU cell (order-2 graph diffusion GRU) Trainium2 Bass kernel, v2.

Strategy: data-parallel over batch (B=16 -> 2 batches per core x 8 cores).
The dominant cost in v1 was streaming the 32MB fp16 adjacency from HBM four
times per core (DMA 99% busy). v2 keeps the whole adjacency RESIDENT in SBUF
as fp8 (x4096 pre-scale keeps the row-normalized values out of e4m3's
denormal range), loaded once (~16MB), and runs all four diffusion passes as
fp8 DoubleRow matmuls (2 packed contraction rows/cycle). Diffused features
are small contributors to the output (the graph averages 4000 nodes), so fp8
error lands ~1e-4 relative; order-k features carry power-of-2 scales folded
into the PSUM-evacuation copies and undone by host-side weight pre-scaling.

Layouts per core: activations node-major fp8 [128p x (chunk, col)] for
diffusion; gate/candidate convs run fp16 from per-band staging tiles
(PE transposes for diffused features, XBAR DMA-transpose from DRAM for the
raw [x;h] features). Gate/candidate nonlinearities on ACT, elementwise on
DVE, combine fused into the last diffusion's band loop.
"""

import numpy as np
import ml_dtypes

import concourse.bass as bass
from concourse import bacc
import concourse.mybir as mybir
import concourse.tile as tile
from concourse.bass_utils import run_bass_kernel_spmd

# problem constants
B, D_IN, D_H, NN = 16, 32, 64, 4000
NCORES = 8
B_LOC = B // NCORES          # batches per core
C = D_IN + D_H               # 96 channels into each gate conv
BC = B_LOC * C               # node-major column count (b-major: [b0 c96 | b1 c96])
BH = B_LOC * D_H             # stacked batch-hidden rows (128)
NP = 4096                    # contraction node dim padded to 32 chunks
CHUNK = 128
NCH = NP // CHUNK            # 32 contraction chunks
NPR = NCH // 2               # 16 DoubleRow chunk pairs
NBAND = 8                    # output-node bands: 7x512 + 416 (= 4000, no pad)
BW = [512] * 7 + [416]
BOFF = [512 * g for g in range(NBAND)]
AOFF = [NCH * 512 * g for g in range(NBAND)]   # at_d col offset per band

F8 = mybir.dt.float8e4
F16 = mybir.dt.float16
F32 = mybir.dt.float32
DR = mybir.MatmulPerfMode.DoubleRow
E4NP = ml_dtypes.float8_e4m3

# fp8 scale chain: adjacency carries x4096 (2^12).
#   z1T carries x32   -> evac scale 32/4096
#   z2T carries x512  -> evac scale 512/(4096*32)
#   zc1 carries x64   -> evac scale 64/4096
#   zc2 stage x512    -> evac scale 512/(4096*64)
S_Z1E, S_Z2E = 2.0 ** -7, 2.0 ** -8
S_C1E, S_C2E = 2.0 ** -6, 2.0 ** -9
# matching host-side weight descales: gate W1 /32, W2 /512; cand x-part
# W1 /32, W2 /512; cand rh-part W1 /64, W2 /512.


def _mlist(g):
    """(offset, width) of the 128-wide m-chunks inside band g."""
    w = BW[g]
    out = []
    mo = 0
    while mo < w:
        out.append((mo, min(CHUNK, w - mo)))
        mo += CHUNK
    return out


def build_program():
    nc = bacc.Bacc("TRN2", target_bir_lowering=False, debug=False)

    at_d = nc.dram_tensor("at", [CHUNK, NCH * NN], F8, kind="ExternalInput").ap()
    zt_d = nc.dram_tensor("zt", [NP, BC], F8, kind="ExternalInput").ap()
    # node-major [x;h] fp16, padded to 128 cols/batch for XBAR dma transpose
    zn_d = nc.dram_tensor("zn", [NP, B_LOC, CHUNK], F16, kind="ExternalInput").ap()
    h_d = nc.dram_tensor("h", [BH, NN], F16, kind="ExternalInput").ap()
    wf_d = nc.dram_tensor("wf", [3, C, D_H], F16, kind="ExternalInput").ap()
    wu_d = nc.dram_tensor("wu", [3, C, D_H], F16, kind="ExternalInput").ap()
    wcx_d = nc.dram_tensor("wcx", [3, D_IN, D_H], F16, kind="ExternalInput").ap()
    wcrh_d = nc.dram_tensor("wcrh", [3, D_H, D_H], F16, kind="ExternalInput").ap()
    bf_d = nc.dram_tensor("bf", [BH, 1], F32, kind="ExternalInput").ap()
    bu_d = nc.dram_tensor("bu", [BH, 1], F32, kind="ExternalInput").ap()
    bc_d = nc.dram_tensor("bcb", [BH, 1], F32, kind="ExternalInput").ap()
    id16_d = nc.dram_tensor("id16", [CHUNK, CHUNK], F16, kind="ExternalInput").ap()
    id8_d = nc.dram_tensor("id8", [CHUNK, CHUNK], F8, kind="ExternalInput").ap()
    out_d = nc.dram_tensor("out", [B_LOC, D_H, NN], F32, kind="ExternalOutput").ap()

    with tile.TileContext(nc) as tc:
        _body(tc, locals())
    nc.compile()
    return nc


def _body(tc, aps):
    nc = tc.nc
    at_d, zt_d, zn_d, h_d = aps["at_d"], aps["zt_d"], aps["zn_d"], aps["h_d"]
    wf_d, wu_d, wcx_d, wcrh_d = (
        aps["wf_d"], aps["wu_d"], aps["wcx_d"], aps["wcrh_d"])
    bf_d, bu_d, bc_d = aps["bf_d"], aps["bu_d"], aps["bc_d"]
    id16_d, id8_d, out_d = aps["id16_d"], aps["id8_d"], aps["out_d"]

    SIG = mybir.ActivationFunctionType.Sigmoid
    TANH = mybir.ActivationFunctionType.Tanh
    COPY = mybir.ActivationFunctionType.Copy

    with (
        tc.tile_pool(name="const", bufs=1) as cpool,
        tc.tile_pool(name="amat", bufs=1) as apool,       # resident adjacency
        tc.tile_pool(name="nm8", bufs=2) as nmpool,       # rotating node-major fp8
        tc.tile_pool(name="perst", bufs=1) as ppool,
        tc.tile_pool(name="stageA", bufs=2) as sApool,    # conv feature stages
        tc.tile_pool(name="stageB", bufs=2) as sBpool,    # wide f16 stages
        tc.tile_pool(name="stageC", bufs=1) as sCpool,    # f32 combine stages
        tc.tile_pool(name="psum", bufs=8, space="PSUM") as pspool,
    ):
        # ---- persistent loads ----
        idm = cpool.tile([CHUNK, CHUNK], F16, tag="idm")
        nc.sync.dma_start(out=idm[:], in_=id16_d[:])
        idm8 = cpool.tile([CHUNK, CHUNK], F8, tag="idm8")
        nc.sync.dma_start(out=idm8[:], in_=id8_d[:])
        wf_sb = [cpool.tile([C, D_H], F16, tag=f"wf{k}", name=f"wf{k}")
                 for k in range(3)]
        wu_sb = [cpool.tile([C, D_H], F16, tag=f"wu{k}", name=f"wu{k}")
                 for k in range(3)]
        wcx_sb = [cpool.tile([D_IN, D_H], F16, tag=f"wcx{k}", name=f"wcx{k}")
                  for k in range(3)]
        wcrh_sb = [cpool.tile([D_H, D_H], F16, tag=f"wcrh{k}", name=f"wcrh{k}")
                   for k in range(3)]
        for k in range(3):
            nc.scalar.dma_start(out=wf_sb[k][:], in_=wf_d[k])
            nc.scalar.dma_start(out=wu_sb[k][:], in_=wu_d[k])
            nc.scalar.dma_start(out=wcx_sb[k][:], in_=wcx_d[k])
            nc.scalar.dma_start(out=wcrh_sb[k][:], in_=wcrh_d[k])
        bf_sb = cpool.tile([BH, 1], F32, tag="bf")
        nc.sync.dma_start(out=bf_sb[:], in_=bf_d[:])
        bu_sb = cpool.tile([BH, 1], F32, tag="bu")
        nc.sync.dma_start(out=bu_sb[:], in_=bu_d[:])
        bc_sb = cpool.tile([BH, 1], F32, tag="bc")
        nc.sync.dma_start(out=bc_sb[:], in_=bc_d[:])

        h_st = ppool.tile([BH, NN], F16, tag="h_st")
        nc.scalar.dma_start(out=h_st[:], in_=h_d[:])

        # node-major [x;h] fp8 for the first diffusion
        ztT = nmpool.tile([CHUNK, NCH * BC], F8, tag="nm", name="ztT")
        nc.sync.dma_start(
            out=ztT[:, :].rearrange("p (j f) -> p j f", j=NCH),
            in_=zt_d[:, :].rearrange("(j p) f -> p j f", p=CHUNK))

        # resident adjacency^T (x4096, fp8), one tile per output band
        at_sb = []
        for g in range(NBAND):
            t = apool.tile([CHUNK, NCH * BW[g]], F8, tag=f"at{g}",
                           name=f"at{g}")
            eng = nc.sync if g % 2 == 0 else nc.scalar
            eng.dma_start(out=t[:], in_=at_d[:, AOFF[g]:AOFF[g] + NCH * BW[g]])
            at_sb.append(t[:, :].rearrange("p (j m) -> p j m", j=NCH))

        u_st = ppool.tile([BH, NN], F16, tag="u_st")
        rh_st = ppool.tile([BH, NP], F16, tag="rh_st")
        nc.vector.memset(rh_st[:, NN:NP], 0.0)
        c_part = ppool.tile([BH, NN], F16, tag="c_part")
        rhT = ppool.tile([CHUNK, NCH * BH], F8, tag="rhT")
        zc1_bm = ppool.tile([BH, NP], F8, tag="zc1_bm")
        nc.vector.memset(zc1_bm[:, NN:NP], 0.0)
        zc1T = ppool.tile([CHUNK, NCH * BH], F8, tag="zc1T")

        zt3 = ztT[:, :].rearrange("p (j f) -> p j f", j=NCH)
        rhT3 = rhT[:, :].rearrange("p (j f) -> p j f", j=NCH)
        zc1T3 = zc1T[:, :].rearrange("p (j f) -> p j f", j=NCH)

        def pass_sa(src3, dstT3, evac_scale):
            """dst = A @ src, node-major -> node-major, adj stationary."""
            for g in range(NBAND):
                ml = _mlist(g)
                pss = [pspool.tile([CHUNK, BC], F32, tag="ps", name=f"psd{mi}")
                       for mi in range(len(ml))]
                for jj in range(NPR):
                    for mi, (mo, mw) in enumerate(ml):
                        nc.tensor.matmul(
                            pss[mi][0:mw, :],
                            lhsT=at_sb[g][:, 2 * jj:2 * jj + 2, mo:mo + mw],
                            rhs=src3[:, 2 * jj:2 * jj + 2, :],
                            start=(jj == 0), stop=(jj == NPR - 1),
                            perf_mode=DR)
                for mi, (mo, mw) in enumerate(ml):
                    nc.scalar.activation(
                        dstT3[0:mw, g * 4 + mi, :], pss[mi][0:mw, :], COPY,
                        scale=evac_scale)

        # ---- phase 1: z1 = A z ----
        z1T = nmpool.tile([CHUNK, NCH * BC], F8, tag="nm", name="z1T")
        z13 = z1T[:, :].rearrange("p (j f) -> p j f", j=NCH)
        nc.vector.memset(z13[:, NCH - 1, :], 0.0)
        pass_sa(zt3, z13, S_Z1E)

        # ---- phase 2: z2 = A z1, fused with gate convs, rh, rhT ----
        z2T = nmpool.tile([CHUNK, NCH * BC], F8, tag="nm", name="z2T")
        z23 = z2T[:, :].rearrange("p (j f) -> p j f", j=NCH)
        nc.vector.memset(z23[:, NCH - 1, :], 0.0)

        for g in range(NBAND):
            ml = _mlist(g)
            m0, w = BOFF[g], BW[g]
            # z2 band
            pss = [pspool.tile([CHUNK, BC], F32, tag="ps", name=f"psd{mi}")
                   for mi in range(len(ml))]
            for jj in range(NPR):
                for mi, (mo, mw) in enumerate(ml):
                    nc.tensor.matmul(
                        pss[mi][0:mw, :],
                        lhsT=at_sb[g][:, 2 * jj:2 * jj + 2, mo:mo + mw],
                        rhs=z13[:, 2 * jj:2 * jj + 2, :],
                        start=(jj == 0), stop=(jj == NPR - 1), perf_mode=DR)
            for mi, (mo, mw) in enumerate(ml):
                nc.scalar.activation(z23[0:mw, g * 4 + mi, :],
                                     pss[mi][0:mw, :], COPY, scale=S_Z2E)

            for b in range(B_LOC):
                rows = slice(b * D_H, (b + 1) * D_H)
                # stage conv features (fp16, base partition 0)
                z0s = sBpool.tile([CHUNK, 512], F16, tag="z0s", name="z0s")
                nc.sync.dma_start_transpose(
                    out=z0s[:, 0:w], in_=zn_d[m0:m0 + w, b, :])
                z1s = sApool.tile([C, 512], F16, tag="z1s", name="z1s")
                z2s = sApool.tile([C, 512], F16, tag="z2s", name="z2s")
                for src3, dst in ((z13, z1s), (z23, z2s)):
                    for mi, (mo, mw) in enumerate(ml):
                        # fp8 PE transpose writes PSUM at element step 2
                        pt = pspool.tile([C, 2 * CHUNK], F8, tag="ps",
                                         name="pt")
                        nc.tensor.transpose(
                            pt[:, 0:2 * CHUNK:2],
                            src3[:, g * 4 + mi, b * C:(b + 1) * C],
                            idm8[:, :])
                        nc.vector.tensor_copy(out=dst[:, mo:mo + mw],
                                              in_=pt[:, 0:2 * mw:2])
                feats = (z0s[0:C, 0:w], z1s[:, 0:w], z2s[:, 0:w])
                feats_x = (z0s[0:D_IN, 0:w], z1s[0:D_IN, 0:w],
                           z2s[0:D_IN, 0:w])
                psf = pspool.tile([BH, 512], F32, tag="ps", name="psf") \
                    if b == 0 else psf
                psu = pspool.tile([BH, 512], F32, tag="ps", name="psu") \
                    if b == 0 else psu
                psx = pspool.tile([BH, 512], F32, tag="ps", name="psx") \
                    if b == 0 else psx
                for k in range(3):
                    nc.tensor.matmul(psf[rows, 0:w], lhsT=wf_sb[k],
                                     rhs=feats[k], start=(k == 0),
                                     stop=(k == 2))
                for k in range(3):
                    nc.tensor.matmul(psu[rows, 0:w], lhsT=wu_sb[k],
                                     rhs=feats[k], start=(k == 0),
                                     stop=(k == 2))
                for k in range(3):
                    nc.tensor.matmul(psx[rows, 0:w], lhsT=wcx_sb[k],
                                     rhs=feats_x[k], start=(k == 0),
                                     stop=(k == 2))
            # gate nonlinearities + rh, full 128 partitions
            rst = sBpool.tile([BH, 512], F16, tag="rst", name="rst")
            nc.scalar.activation(rst[:, 0:w], psf[:, 0:w], SIG, bias=bf_sb[:, :])
            nc.scalar.activation(u_st[:, m0:m0 + w], psu[:, 0:w], SIG,
                                 bias=bu_sb[:, :])
            nc.vector.tensor_mul(out=rh_st[:, m0:m0 + w], in0=rst[:, 0:w],
                                 in1=h_st[:, m0:m0 + w])
            nc.vector.tensor_copy(out=c_part[:, m0:m0 + w], in_=psx[:, 0:w])
            # rhT for the candidate diffusion (node-major fp8)
            for b in range(B_LOC):
                rows = slice(b * D_H, (b + 1) * D_H)
                for mi, (mo, mw) in enumerate(ml):
                    ch = g * 4 + mi
                    ptr = pspool.tile([CHUNK, D_H], F16, tag="ps", name="ptr")
                    nc.tensor.transpose(
                        ptr[:, :],
                        rh_st[rows, ch * CHUNK:(ch + 1) * CHUNK],
                        idm[rows, rows])
                    nc.vector.tensor_copy(
                        out=rhT3[:, ch, b * D_H:(b + 1) * D_H], in_=ptr[:, :])

        # ---- phase 3: zc1 = A rh (activations stationary, adj moving) ----
        for g in range(NBAND):
            m0, w = BOFF[g], BW[g]
            psc = pspool.tile([BH, 512], F32, tag="ps", name="psc")
            for jj in range(NPR):
                nc.tensor.matmul(
                    psc[:, 0:w],
                    lhsT=rhT3[:, 2 * jj:2 * jj + 2, :],
                    rhs=at_sb[g][:, 2 * jj:2 * jj + 2, 0:w],
                    start=(jj == 0), stop=(jj == NPR - 1), perf_mode=DR)
            nc.scalar.activation(zc1_bm[:, m0:m0 + w], psc[:, 0:w], COPY,
                                 scale=S_C1E)
            for b in range(B_LOC):
                rows = slice(b * D_H, (b + 1) * D_H)
                for mi, (mo, mw) in enumerate(_mlist(g)):
                    ch = g * 4 + mi
                    ptc = pspool.tile([CHUNK, 2 * D_H], F8, tag="ps",
                                      name="ptc")
                    nc.tensor.transpose(
                        ptc[:, 0:2 * D_H:2],
                        zc1_bm[rows, ch * CHUNK:(ch + 1) * CHUNK],
                        idm8[rows, rows])
                    nc.vector.tensor_copy(
                        out=zc1T3[:, ch, b * D_H:(b + 1) * D_H],
                        in_=ptc[:, 0:2 * D_H:2])

        # ---- phase 4: zc2 = A zc1, fused candidate conv + combine ----
        for g in range(NBAND):
            m0, w = BOFF[g], BW[g]
            psc2 = pspool.tile([BH, 512], F32, tag="ps", name="psc2")
            for jj in range(NPR):
                nc.tensor.matmul(
                    psc2[:, 0:w],
                    lhsT=zc1T3[:, 2 * jj:2 * jj + 2, :],
                    rhs=at_sb[g][:, 2 * jj:2 * jj + 2, 0:w],
                    start=(jj == 0), stop=(jj == NPR - 1), perf_mode=DR)
            zc2s = sBpool.tile([BH, 512], F16, tag="zc2s", name="zc2s")
            nc.scalar.activation(zc2s[:, 0:w], psc2[:, 0:w], COPY,
                                 scale=S_C2E)
            zc1s = sBpool.tile([BH, 512], F16, tag="zc1s", name="zc1s")
            nc.vector.tensor_copy(out=zc1s[:, 0:w], in_=zc1_bm[:, m0:m0 + w])
            # batch-1 features need base partition 0: SBUF->SBUF DMA restage
            b1rh = sApool.tile([D_H, 512], F16, tag="b1rh", name="b1rh")
            nc.scalar.dma_start(out=b1rh[:, 0:w], in_=rh_st[D_H:BH, m0:m0 + w])
            b1c1 = sApool.tile([D_H, 512], F16, tag="b1c1", name="b1c1")
            nc.scalar.dma_start(out=b1c1[:, 0:w], in_=zc1s[D_H:BH, 0:w])
            b1c2 = sApool.tile([D_H, 512], F16, tag="b1c2", name="b1c2")
            nc.scalar.dma_start(out=b1c2[:, 0:w], in_=zc2s[D_H:BH, 0:w])
            psc3 = pspool.tile([BH, 512], F32, tag="ps", name="psc3")
            for b in range(B_LOC):
                rows = slice(b * D_H, (b + 1) * D_H)
                terms = ((rh_st[0:D_H, m0:m0 + w], zc1s[0:D_H, 0:w],
                          zc2s[0:D_H, 0:w]) if b == 0 else
                         (b1rh[:, 0:w], b1c1[:, 0:w], b1c2[:, 0:w]))
                for k in range(3):
                    nc.tensor.matmul(psc3[rows, 0:w], lhsT=wcrh_sb[k],
                                     rhs=terms[k], start=(k == 0),
                                     stop=(k == 2))
            tt = sCpool.tile([BH, 512], F32, tag="tt", name="tt")
            nc.vector.tensor_add(out=tt[:, 0:w], in0=psc3[:, 0:w],
                                 in1=c_part[:, m0:m0 + w])
            cst = sCpool.tile([BH, 512], F32, tag="cst", name="cst")
            nc.scalar.activation(cst[:, 0:w], tt[:, 0:w], TANH, bias=bc_sb[:, :])
            t1 = sCpool.tile([BH, 512], F32, tag="t1", name="t1")
            nc.vector.tensor_sub(out=t1[:, 0:w], in0=h_st[:, m0:m0 + w],
                                 in1=cst[:, 0:w])
            nc.vector.tensor_mul(out=t1[:, 0:w], in0=u_st[:, m0:m0 + w],
                                 in1=t1[:, 0:w])
            nc.vector.tensor_add(out=cst[:, 0:w], in0=cst[:, 0:w],
                                 in1=t1[:, 0:w])
            for b in range(B_LOC):
                nc.sync.dma_start(
                    out=out_d[b][:, m0:m0 + w],
                    in_=cst[b * D_H:(b + 1) * D_H, 0:w])


# ---- host-side driver ----
_CACHED_NC = None
TRACE = False           # set True (e.g. from test.py) to capture an NTFF profile
TRACE_DIR = None
LAST_RESULTS = None     # BassKernelResults of the most recent kernel() call


def _host_prep(x, h, adj, Wf, bf, Wu, bu, Wc, bc):
    """Shard + cast + layout inputs for the 8 cores. Returns list of in_maps."""
    atp = np.zeros((NP, NN), dtype=np.float32)
    atp[:NN] = adj.T * 4096.0
    at8 = atp.astype(E4NP)                       # [4096, 4000]
    blocks = at8.reshape(NCH, CHUNK, NN)
    cols = [np.ascontiguousarray(
        blocks[:, :, BOFF[g]:BOFF[g] + BW[g]].transpose(1, 0, 2)
    ).reshape(CHUNK, NCH * BW[g]) for g in range(NBAND)]
    at_h = np.ascontiguousarray(np.concatenate(cols, axis=1))

    id16 = np.eye(CHUNK, dtype=np.float16)
    id8 = np.eye(CHUNK, dtype=E4NP)

    wsc = {"wf": (1.0, 1 / 32., 1 / 512.), "wu": (1.0, 1 / 32., 1 / 512.),
           "wcx": (1.0, 1 / 32., 1 / 512.), "wcrh": (1.0, 1 / 64., 1 / 512.)}

    def wsplit(W, key, lo, hi):
        return np.ascontiguousarray(np.stack(
            [(W[:, k * C + lo:k * C + hi].T * wsc[key][k]).astype(np.float16)
             for k in range(3)]))

    def bstack(v):
        return np.concatenate([v] * B_LOC).reshape(BH, 1).astype(np.float32)

    shared = {
        "at": at_h, "id16": id16, "id8": id8,
        "wf": wsplit(Wf, "wf", 0, C), "wu": wsplit(Wu, "wu", 0, C),
        "wcx": wsplit(Wc, "wcx", 0, D_IN), "wcrh": wsplit(Wc, "wcrh", D_IN, C),
        "bf": bstack(bf), "bu": bstack(bu), "bcb": bstack(bc),
    }
    in_maps = []
    for core in range(NCORES):
        bs = slice(core * B_LOC, (core + 1) * B_LOC)
        z = np.concatenate([x[bs], h[bs]], axis=1)       # [B_LOC, C, NN]
        znm = z.transpose(2, 0, 1)                       # [NN, B_LOC, C]
        ztp = np.zeros((NP, BC), dtype=np.float32)
        ztp[:NN] = znm.reshape(NN, BC)
        znp = np.zeros((NP, B_LOC, CHUNK), dtype=np.float16)
        znp[:NN, :, :C] = znm
        h_p = np.ascontiguousarray(
            h[bs].astype(np.float16).reshape(BH, NN))
        in_maps.append(dict(shared, zt=ztp.astype(E4NP), zn=znp, h=h_p))
    return in_maps


def kernel(**inputs):
    global _CACHED_NC, LAST_RESULTS
    inputs = {k: np.asarray(v) for k, v in inputs.items()}
    if _CACHED_NC is None:
        _CACHED_NC = build_program()
    in_maps = _host_prep(**inputs)
    kw = {}
    if TRACE:
        kw = dict(trace=True, tmpdir=TRACE_DIR)
    res = run_bass_kernel_spmd(_CACHED_NC, in_maps,
                               core_ids=list(range(NCORES)), **kw)
    LAST_RESULTS = res
    outs = [res.results[i]["out"] for i in range(NCORES)]
    return np.concatenate(outs, axis=0).astype(np.float32)


if __name__ == "__main__":
    rng = np.random.default_rng(0)
    ins = {
        "x": rng.standard_normal((B, D_IN, NN), dtype=np.float32),
        "h": rng.standard_normal((B, D_H, NN), dtype=np.float32),
        "adj": rng.random((NN, NN), dtype=np.float32) / NN,
        "Wf": rng.standard_normal((D_H, 3 * C), dtype=np.float32) * 0.05,
        "Wu": rng.standard_normal((D_H, 3 * C), dtype=np.float32) * 0.05,
        "Wc": rng.standard_normal((D_H, 3 * C), dtype=np.float32) * 0.05,
        "bf": rng.standard_normal(D_H).astype(np.float32) * 0.05,
        "bu": rng.standard_normal(D_H).astype(np.float32) * 0.05,
        "bc": rng.standard_normal(D_H).astype(np.float32) * 0.05,
    }
    out = kernel(**ins)
    print(out.shape, out.dtype)


# revision 12
# speedup vs baseline: 1.4586x; 1.0058x over previous
"""GCGRU cell (order-2 graph diffusion GRU) Trainium2 Bass kernel, v2.

Strategy: data-parallel over batch (B=16 -> 2 batches per core x 8 cores).
The dominant cost in v1 was streaming the 32MB fp16 adjacency from HBM four
times per core (DMA 99% busy). v2 keeps the whole adjacency RESIDENT in SBUF
as fp8 (x4096 pre-scale keeps the row-normalized values out of e4m3's
denormal range), loaded once (~16MB), and runs all four diffusion passes as
fp8 DoubleRow matmuls (2 packed contraction rows/cycle). Diffused features
are small contributors to the output (the graph averages 4000 nodes), so fp8
error lands ~1e-4 relative; order-k features carry power-of-2 scales folded
into the PSUM-evacuation copies and undone by host-side weight pre-scaling.

Layouts per core: activations node-major fp8 [128p x (chunk, col)] for
diffusion; gate/candidate convs run fp16 from per-band staging tiles
(PE transposes for diffused features, XBAR DMA-transpose from DRAM for the
raw [x;h] features). Gate/candidate nonlinearities on ACT, elementwise on
DVE, combine fused into the last diffusion's band loop.
"""

import numpy as np
import ml_dtypes

import concourse.bass as bass
from concourse import bacc
import concourse.mybir as mybir
import concourse.tile as tile
from concourse.bass_utils import run_bass_kernel_spmd

# problem constants
B, D_IN, D_H, NN = 16, 32, 64, 4000
NCORES = 8
B_LOC = B // NCORES          # batches per core
C = D_IN + D_H               # 96 channels into each gate conv
BC = B_LOC * C               # node-major column count (b-major: [b0 c96 | b1 c96])
BH = B_LOC * D_H             # stacked batch-hidden rows (128)
NP = 4096                    # contraction node dim padded to 32 chunks
CHUNK = 128
NCH = NP // CHUNK            # 32 contraction chunks
NPR = NCH // 2               # 16 DoubleRow chunk pairs
NBAND = 8                    # output-node bands: 7x512 + 416 (= 4000, no pad)
BW = [512] * 7 + [416]
BOFF = [512 * g for g in range(NBAND)]
AOFF = [NCH * 512 * g for g in range(NBAND)]   # at_d col offset per band

F8 = mybir.dt.float8e4
F16 = mybir.dt.float16
F32 = mybir.dt.float32
DR = mybir.MatmulPerfMode.DoubleRow
E4NP = ml_dtypes.float8_e4m3

# fp8 scale chain: adjacency carries x4096 (2^12).
#   z1T carries x32   -> evac scale 32/4096
#   z2T carries x512  -> evac scale 512/(4096*32)
#   zc1 carries x64   -> evac scale 64/4096
#   zc2 stage x512    -> evac scale 512/(4096*64)
S_Z1E, S_Z2E = 2.0 ** -7, 2.0 ** -8
S_C1E, S_C2E = 2.0 ** -6, 2.0 ** -9
# matching host-side weight descales: gate W1 /32, W2 /512; cand x-part
# W1 /32, W2 /512; cand rh-part W1 /64, W2 /512.


def _mlist(g):
    """(offset, width) of the 128-wide m-chunks inside band g."""
    w = BW[g]
    out = []
    mo = 0
    while mo < w:
        out.append((mo, min(CHUNK, w - mo)))
        mo += CHUNK
    return out


def build_program():
    nc = bacc.Bacc("TRN2", target_bir_lowering=False, debug=False)

    at_d = nc.dram_tensor("at", [CHUNK, NCH * NN], F8, kind="ExternalInput").ap()
    zt_d = nc.dram_tensor("zt", [NP, BC], F8, kind="ExternalInput").ap()
    # node-major [x;h] fp16, padded to 128 cols/batch for XBAR dma transpose
    zn_d = nc.dram_tensor("zn", [NP, B_LOC, CHUNK], F16, kind="ExternalInput").ap()
    h_d = nc.dram_tensor("h", [BH, NN], F16, kind="ExternalInput").ap()
    wf_d = nc.dram_tensor("wf", [3, C, D_H], F16, kind="ExternalInput").ap()
    wu_d = nc.dram_tensor("wu", [3, C, D_H], F16, kind="ExternalInput").ap()
    wcx_d = nc.dram_tensor("wcx", [3, D_IN, D_H], F16, kind="ExternalInput").ap()
    wcrh_d = nc.dram_tensor("wcrh", [3, D_H, D_H], F16, kind="ExternalInput").ap()
    bf_d = nc.dram_tensor("bf", [BH, 1], F32, kind="ExternalInput").ap()
    bu_d = nc.dram_tensor("bu", [BH, 1], F32, kind="ExternalInput").ap()
    bc_d = nc.dram_tensor("bcb", [BH, 1], F32, kind="ExternalInput").ap()
    id16_d = nc.dram_tensor("id16", [CHUNK, CHUNK], F16, kind="ExternalInput").ap()
    id8_d = nc.dram_tensor("id8", [CHUNK, CHUNK], F8, kind="ExternalInput").ap()
    out_d = nc.dram_tensor("out", [B_LOC, D_H, NN], F32, kind="ExternalOutput").ap()

    with tile.TileContext(nc) as tc:
        _body(tc, locals())
    nc.compile()
    return nc


def _body(tc, aps):
    nc = tc.nc
    at_d, zt_d, zn_d, h_d = aps["at_d"], aps["zt_d"], aps["zn_d"], aps["h_d"]
    wf_d, wu_d, wcx_d, wcrh_d = (
        aps["wf_d"], aps["wu_d"], aps["wcx_d"], aps["wcrh_d"])
    bf_d, bu_d, bc_d = aps["bf_d"], aps["bu_d"], aps["bc_d"]
    id16_d, id8_d, out_d = aps["id16_d"], aps["id8_d"], aps["out_d"]

    SIG = mybir.ActivationFunctionType.Sigmoid
    TANH = mybir.ActivationFunctionType.Tanh
    COPY = mybir.ActivationFunctionType.Copy

    with (
        tc.tile_pool(name="const", bufs=1) as cpool,
        tc.tile_pool(name="amat", bufs=1) as apool,       # resident adjacency
        tc.tile_pool(name="nm8", bufs=2) as nmpool,       # rotating node-major fp8
        tc.tile_pool(name="perst", bufs=1) as ppool,
        tc.tile_pool(name="stageA", bufs=2) as sApool,    # conv feature stages
        tc.tile_pool(name="stageB", bufs=2) as sBpool,    # wide f16 stages
        tc.tile_pool(name="stageC", bufs=2) as sCpool,    # f32 combine stages
        tc.tile_pool(name="psum", bufs=8, space="PSUM") as pspool,
    ):
        # ---- persistent loads ----
        # DMA priority: phase 1 is gated on ztT + at0, so those go first on
        # separate rings; weights/h/idm are not needed until phase 2.
        ztT = nmpool.tile([CHUNK, NCH * BC], F8, tag="nm", name="ztT")
        nc.scalar.dma_start(
            out=ztT[:, :].rearrange("p (j f) -> p j f", j=NCH),
            in_=zt_d[:, :].rearrange("(j p) f -> p j f", p=CHUNK))
        # resident adjacency^T (x4096, fp8), one tile per output band
        at_sb = []
        for g in range(NBAND):
            t = apool.tile([CHUNK, NCH * BW[g]], F8, tag=f"at{g}",
                           name=f"at{g}")
            eng = nc.sync if g % 2 == 0 else nc.scalar
            eng.dma_start(out=t[:], in_=at_d[:, AOFF[g]:AOFF[g] + NCH * BW[g]])
            at_sb.append(t[:, :].rearrange("p (j m) -> p j m", j=NCH))

        idm = cpool.tile([CHUNK, CHUNK], F16, tag="idm")
        nc.sync.dma_start(out=idm[:], in_=id16_d[:])
        idm8 = cpool.tile([CHUNK, CHUNK], F8, tag="idm8")
        nc.sync.dma_start(out=idm8[:], in_=id8_d[:])
        wf_sb = [cpool.tile([C, D_H], F16, tag=f"wf{k}", name=f"wf{k}")
                 for k in range(3)]
        wu_sb = [cpool.tile([C, D_H], F16, tag=f"wu{k}", name=f"wu{k}")
                 for k in range(3)]
        wcx_sb = [cpool.tile([D_IN, D_H], F16, tag=f"wcx{k}", name=f"wcx{k}")
                  for k in range(3)]
        wcrh_sb = [cpool.tile([D_H, D_H], F16, tag=f"wcrh{k}", name=f"wcrh{k}")
                   for k in range(3)]
        for k in range(3):
            nc.scalar.dma_start(out=wf_sb[k][:], in_=wf_d[k])
            nc.scalar.dma_start(out=wu_sb[k][:], in_=wu_d[k])
            nc.scalar.dma_start(out=wcx_sb[k][:], in_=wcx_d[k])
            nc.scalar.dma_start(out=wcrh_sb[k][:], in_=wcrh_d[k])
        bf_sb = cpool.tile([BH, 1], F32, tag="bf")
        nc.sync.dma_start(out=bf_sb[:], in_=bf_d[:])
        bu_sb = cpool.tile([BH, 1], F32, tag="bu")
        nc.sync.dma_start(out=bu_sb[:], in_=bu_d[:])
        bc_sb = cpool.tile([BH, 1], F32, tag="bc")
        nc.sync.dma_start(out=bc_sb[:], in_=bc_d[:])

        h_st = ppool.tile([BH, NN], F16, tag="h_st")
        nc.scalar.dma_start(out=h_st[:], in_=h_d[:])

        u_st = ppool.tile([BH, NN], F16, tag="u_st")
        rh_st = ppool.tile([BH, NP], F16, tag="rh_st")
        nc.vector.memset(rh_st[:, NN:NP], 0.0)
        c_part = ppool.tile([BH, NN], F16, tag="c_part")
        rhT = ppool.tile([CHUNK, NCH * BH], F8, tag="rhT")
        zc1_bm = ppool.tile([BH, NP], F8, tag="zc1_bm")
        nc.vector.memset(zc1_bm[:, NN:NP], 0.0)
        zc1T = ppool.tile([CHUNK, NCH * BH], F8, tag="zc1T")

        zt3 = ztT[:, :].rearrange("p (j f) -> p j f", j=NCH)
        rhT3 = rhT[:, :].rearrange("p (j f) -> p j f", j=NCH)
        zc1T3 = zc1T[:, :].rearrange("p (j f) -> p j f", j=NCH)

        def pass_sa(src3, dstT3, evac_scale):
            """dst = A @ src, node-major -> node-major, adj stationary."""
            for g in range(NBAND):
                ml = _mlist(g)
                pss = [pspool.tile([CHUNK, BC], F32, tag="ps", name=f"psd{mi}")
                       for mi in range(len(ml))]
                for jj in range(NPR):
                    for mi, (mo, mw) in enumerate(ml):
                        nc.tensor.matmul(
                            pss[mi][0:mw, :],
                            lhsT=at_sb[g][:, 2 * jj:2 * jj + 2, mo:mo + mw],
                            rhs=src3[:, 2 * jj:2 * jj + 2, :],
                            start=(jj == 0), stop=(jj == NPR - 1),
                            perf_mode=DR)
                for mi, (mo, mw) in enumerate(ml):
                    nc.scalar.activation(
                        dstT3[0:mw, g * 4 + mi, :], pss[mi][0:mw, :], COPY,
                        scale=evac_scale)

        # ---- phase 1: z1 = A z ----
        z1T = nmpool.tile([CHUNK, NCH * BC], F8, tag="nm", name="z1T")
        z13 = z1T[:, :].rearrange("p (j f) -> p j f", j=NCH)
        nc.vector.memset(z13[:, NCH - 1, :], 0.0)
        pass_sa(zt3, z13, S_Z1E)

        # ---- phase 2: z2 = A z1, fused with gate convs, rh, rhT ----
        z2T = nmpool.tile([CHUNK, NCH * BC], F8, tag="nm", name="z2T")
        z23 = z2T[:, :].rearrange("p (j f) -> p j f", j=NCH)
        nc.vector.memset(z23[:, NCH - 1, :], 0.0)

        for g in range(NBAND):
            ml = _mlist(g)
            m0, w = BOFF[g], BW[g]
            # z2 band
            pss = [pspool.tile([CHUNK, BC], F32, tag="ps", name=f"psd{mi}")
                   for mi in range(len(ml))]
            for jj in range(NPR):
                for mi, (mo, mw) in enumerate(ml):
                    nc.tensor.matmul(
                        pss[mi][0:mw, :],
                        lhsT=at_sb[g][:, 2 * jj:2 * jj + 2, mo:mo + mw],
                        rhs=z13[:, 2 * jj:2 * jj + 2, :],
                        start=(jj == 0), stop=(jj == NPR - 1), perf_mode=DR)
            for mi, (mo, mw) in enumerate(ml):
                nc.scalar.activation(z23[0:mw, g * 4 + mi, :],
                                     pss[mi][0:mw, :], COPY, scale=S_Z2E)

            for b in range(B_LOC):
                rows = slice(b * D_H, (b + 1) * D_H)
                # stage conv features (fp16, base partition 0)
                z0s = sBpool.tile([CHUNK, 512], F16, tag="z0s", name="z0s")
                nc.sync.dma_start_transpose(
                    out=z0s[:, 0:w], in_=zn_d[m0:m0 + w, b, :])
                z1s = sApool.tile([C, 512], F16, tag="z1s", name="z1s")
                z2s = sApool.tile([C, 512], F16, tag="z2s", name="z2s")
                for src3, dst in ((z13, z1s), (z23, z2s)):
                    for mi, (mo, mw) in enumerate(ml):
                        # fp8 PE transpose writes PSUM at element step 2
                        pt = pspool.tile([C, 2 * CHUNK], F8, tag="ps",
                                         name="pt")
                        nc.tensor.transpose(
                            pt[:, 0:2 * CHUNK:2],
                            src3[:, g * 4 + mi, b * C:(b + 1) * C],
                            idm8[:, :])
                        nc.vector.tensor_copy(out=dst[:, mo:mo + mw],
                                              in_=pt[:, 0:2 * mw:2])
                feats = (z0s[0:C, 0:w], z1s[:, 0:w], z2s[:, 0:w])
                feats_x = (z0s[0:D_IN, 0:w], z1s[0:D_IN, 0:w],
                           z2s[0:D_IN, 0:w])
                psf = pspool.tile([BH, 512], F32, tag="ps", name="psf") \
                    if b == 0 else psf
                psu = pspool.tile([BH, 512], F32, tag="ps", name="psu") \
                    if b == 0 else psu
                psx = pspool.tile([BH, 512], F32, tag="ps", name="psx") \
                    if b == 0 else psx
                for k in range(3):
                    nc.tensor.matmul(psf[rows, 0:w], lhsT=wf_sb[k],
                                     rhs=feats[k], start=(k == 0),
                                     stop=(k == 2))
                for k in range(3):
                    nc.tensor.matmul(psu[rows, 0:w], lhsT=wu_sb[k],
                                     rhs=feats[k], start=(k == 0),
                                     stop=(k == 2))
                for k in range(3):
                    nc.tensor.matmul(psx[rows, 0:w], lhsT=wcx_sb[k],
                                     rhs=feats_x[k], start=(k == 0),
                                     stop=(k == 2))
            # gate nonlinearities + rh, full 128 partitions
            rst = sBpool.tile([BH, 512], F16, tag="rst", name="rst")
            nc.scalar.activation(rst[:, 0:w], psf[:, 0:w], SIG, bias=bf_sb[:, :])
            nc.scalar.activation(u_st[:, m0:m0 + w], psu[:, 0:w], SIG,
                                 bias=bu_sb[:, :])
            nc.vector.tensor_mul(out=rh_st[:, m0:m0 + w], in0=rst[:, 0:w],
                                 in1=h_st[:, m0:m0 + w])
            nc.vector.tensor_copy(out=c_part[:, m0:m0 + w], in_=psx[:, 0:w])
            # rhT for the candidate diffusion (node-major fp8)
            for b in range(B_LOC):
                rows = slice(b * D_H, (b + 1) * D_H)
                for mi, (mo, mw) in enumerate(ml):
                    ch = g * 4 + mi
                    ptr = pspool.tile([CHUNK, D_H], F16, tag="ps", name="ptr")
                    nc.tensor.transpose(
                        ptr[:, :],
                        rh_st[rows, ch * CHUNK:(ch + 1) * CHUNK],
                        idm[rows, rows])
                    nc.vector.tensor_copy(
                        out=rhT3[:, ch, b * D_H:(b + 1) * D_H], in_=ptr[:, :])

        # ---- phase 3: zc1 = A rh (activations stationary, adj moving) ----
        for g in range(NBAND):
            m0, w = BOFF[g], BW[g]
            psc = pspool.tile([BH, 512], F32, tag="ps", name="psc")
            for jj in range(NPR):
                nc.tensor.matmul(
                    psc[:, 0:w],
                    lhsT=rhT3[:, 2 * jj:2 * jj + 2, :],
                    rhs=at_sb[g][:, 2 * jj:2 * jj + 2, 0:w],
                    start=(jj == 0), stop=(jj == NPR - 1), perf_mode=DR)
            nc.scalar.activation(zc1_bm[:, m0:m0 + w], psc[:, 0:w], COPY,
                                 scale=S_C1E)
            for b in range(B_LOC):
                rows = slice(b * D_H, (b + 1) * D_H)
                for mi, (mo, mw) in enumerate(_mlist(g)):
                    ch = g * 4 + mi
                    ptc = pspool.tile([CHUNK, 2 * D_H], F8, tag="ps",
                                      name="ptc")
                    nc.tensor.transpose(
                        ptc[:, 0:2 * D_H:2],
                        zc1_bm[rows, ch * CHUNK:(ch + 1) * CHUNK],
                        idm8[rows, rows])
                    nc.vector.tensor_copy(
                        out=zc1T3[:, ch, b * D_H:(b + 1) * D_H],
                        in_=ptc[:, 0:2 * D_H:2])

        # ---- phase 4: zc2 = A zc1, fused candidate conv + combine ----
        for g in range(NBAND):
            m0, w = BOFF[g], BW[g]
            psc2 = pspool.tile([BH, 512], F32, tag="ps", name="psc2")
            for jj in range(NPR):
                nc.tensor.matmul(
                    psc2[:, 0:w],
                    lhsT=zc1T3[:, 2 * jj:2 * jj + 2, :],
                    rhs=at_sb[g][:, 2 * jj:2 * jj + 2, 0:w],
                    start=(jj == 0), stop=(jj == NPR - 1), perf_mode=DR)
            zc2s = sBpool.tile([BH, 512], F16, tag="zc2s", name="zc2s")
            nc.scalar.activation(zc2s[:, 0:w], psc2[:, 0:w], COPY,
                                 scale=S_C2E)
            zc1s = sBpool.tile([BH, 512], F16, tag="zc1s", name="zc1s")
            nc.vector.tensor_copy(out=zc1s[:, 0:w], in_=zc1_bm[:, m0:m0 + w])
            # batch-1 features need base partition 0: SBUF->SBUF DMA restage
            b1rh = sApool.tile([D_H, 512], F16, tag="b1rh", name="b1rh")
            nc.scalar.dma_start(out=b1rh[:, 0:w], in_=rh_st[D_H:BH, m0:m0 + w])
            b1c1 = sApool.tile([D_H, 512], F16, tag="b1c1", name="b1c1")
            nc.scalar.dma_start(out=b1c1[:, 0:w], in_=zc1s[D_H:BH, 0:w])
            b1c2 = sApool.tile([D_H, 512], F16, tag="b1c2", name="b1c2")
            nc.scalar.dma_start(out=b1c2[:, 0:w], in_=zc2s[D_H:BH, 0:w])
            psc3 = pspool.tile([BH, 512], F32, tag="ps", name="psc3")
            for b in range(B_LOC):
                rows = slice(b * D_H, (b + 1) * D_H)
                terms = ((rh_st[0:D_H, m0:m0 + w], zc1s[0:D_H, 0:w],
                          zc2s[0:D_H, 0:w]) if b == 0 else
                         (b1rh[:, 0:w], b1c1[:, 0:w], b1c2[:, 0:w]))
                for k in range(3):
                    nc.tensor.matmul(psc3[rows, 0:w], lhsT=wcrh_sb[k],
                                     rhs=terms[k], start=(k == 0),
                                     stop=(k == 2))
            tt = sCpool.tile([BH, 512], F16, tag="tt", name="tt")
            nc.vector.tensor_add(out=tt[:, 0:w], in0=psc3[:, 0:w],
                                 in1=c_part[:, m0:m0 + w])
            cst = sCpool.tile([BH, 512], F32, tag="cst", name="cst")
            nc.scalar.activation(cst[:, 0:w], tt[:, 0:w], TANH, bias=bc_sb[:, :])
            # combine on the otherwise-idle Pool engine; tt reused for h-c
            nc.gpsimd.tensor_sub(out=tt[:, 0:w], in0=h_st[:, m0:m0 + w],
                                 in1=cst[:, 0:w])
            nc.gpsimd.tensor_mul(out=tt[:, 0:w], in0=u_st[:, m0:m0 + w],
                                 in1=tt[:, 0:w])
            nc.gpsimd.tensor_add(out=cst[:, 0:w], in0=cst[:, 0:w],
                                 in1=tt[:, 0:w])
            for b in range(B_LOC):
                nc.sync.dma_start(
                    out=out_d[b][:, m0:m0 + w],
                    in_=cst[b * D_H:(b + 1) * D_H, 0:w])


# ---- host-side driver ----
_CACHED_NC = None
TRACE = False           # set True (e.g. from test.py) to capture an NTFF profile
TRACE_DIR = None
LAST_RESULTS = None     # BassKernelResults of the most recent kernel() call


def _host_prep(x, h, adj, Wf, bf, Wu, bu, Wc, bc):
    """Shard + cast + layout inputs for the 8 cores. Returns list of in_maps."""
    atp = np.zeros((NP, NN), dtype=np.float32)
    atp[:NN] = adj.T * 4096.0
    at8 = atp.astype(E4NP)                       # [4096, 4000]
    blocks = at8.reshape(NCH, CHUNK, NN)
    cols = [np.ascontiguousarray(
        blocks[:, :, BOFF[g]:BOFF[g] + BW[g]].transpose(1, 0, 2)
    ).reshape(CHUNK, NCH * BW[g]) for g in range(NBAND)]
    at_h = np.ascontiguousarray(np.concatenate(cols, axis=1))

    id16 = np.eye(CHUNK, dtype=np.float16)
    id8 = np.eye(CHUNK, dtype=E4NP)

    wsc = {"wf": (1.0, 1 / 32., 1 / 512.), "wu": (1.0, 1 / 32., 1 / 512.),
           "wcx": (1.0, 1 / 32., 1 / 512.), "wcrh": (1.0, 1 / 64., 1 / 512.)}

    def wsplit(W, key, lo, hi):
        return np.ascontiguousarray(np.stack(
            [(W[:, k * C + lo:k * C + hi].T * wsc[key][k]).astype(np.float16)
             for k in range(3)]))

    def bstack(v):
        return np.concatenate([v] * B_LOC).reshape(BH, 1).astype(np.float32)

    shared = {
        "at": at_h, "id16": id16, "id8": id8,
        "wf": wsplit(Wf, "wf", 0, C), "wu": wsplit(Wu, "wu", 0, C),
        "wcx": wsplit(Wc, "wcx", 0, D_IN), "wcrh": wsplit(Wc, "wcrh", D_IN, C),
        "bf": bstack(bf), "bu": bstack(bu), "bcb": bstack(bc),
    }
    in_maps = []
    for core in range(NCORES):
        bs = slice(core * B_LOC, (core + 1) * B_LOC)
        z = np.concatenate([x[bs], h[bs]], axis=1)       # [B_LOC, C, NN]
        znm = z.transpose(2, 0, 1)                       # [NN, B_LOC, C]
        ztp = np.zeros((NP, BC), dtype=np.float32)
        ztp[:NN] = znm.reshape(NN, BC)
        znp = np.zeros((NP, B_LOC, CHUNK), dtype=np.float16)
        znp[:NN, :, :C] = znm
        h_p = np.ascontiguousarray(
            h[bs].astype(np.float16).reshape(BH, NN))
        in_maps.append(dict(shared, zt=ztp.astype(E4NP), zn=znp, h=h_p))
    return in_maps


def kernel(**inputs):
    global _CACHED_NC, LAST_RESULTS
    inputs = {k: np.asarray(v) for k, v in inputs.items()}
    if _CACHED_NC is None:
        _CACHED_NC = build_program()
    in_maps = _host_prep(**inputs)
    kw = {}
    if TRACE:
        kw = dict(trace=True, tmpdir=TRACE_DIR)
    res = run_bass_kernel_spmd(_CACHED_NC, in_maps,
                               core_ids=list(range(NCORES)), **kw)
    LAST_RESULTS = res
    outs = [res.results[i]["out"] for i in range(NCORES)]
    return np.concatenate(outs, axis=0).astype(np.float32)


if __name__ == "__main__":
    rng = np.random.default_rng(0)
    ins = {
        "x": rng.standard_normal((B, D_IN, NN), dtype=np.float32),
        "h": rng.standard_normal((B, D_H, NN), dtype=np.float32),
        "adj": rng.random((NN, NN), dtype=np.float32) / NN,
        "Wf": rng.standard_normal((D_H, 3 * C), dtype=np.float32) * 0.05,
        "Wu": rng.standard_normal((D_H, 3 * C), dtype=np.float32) * 0.05,
        "Wc": rng.standard_normal((D_H, 3 * C), dtype=np.float32) * 0.05,
        "bf": rng.standard_normal(D_H).astype(np.float32) * 0.05,
        "bu": rng.standard_normal(D_H).astype(np.float32) * 0.05,
        "bc": rng.standard_normal(D_H).astype(np.float32) * 0.05,
    }
    out = kernel(**ins)
    print(out.shape, out.dtype)
